# revision 3
# baseline (speedup 1.0000x reference)
"""MoE (8 experts, top-2 routing) kernel for Trainium2 — hidden-dim-sharded
(tensor-parallel) across 8 NeuronCores, all matmuls in bf16.

Why hidden-shard instead of expert-parallel: with one expert per core the
slowest core pads its token group to the global max (1152 of a 1024 mean),
wasting ~12% of the PE. Sharding the H=4096 hidden dim instead gives every
core a 512-wide slice of ALL 8 experts' W1/W2, so all cores do the exact
same amount of work (the full 2T = 8192 routed (token, expert) pairs at
1/8 the hidden width each), with zero token padding: both matmuls keep
tokens on the moving dim, which can be any size.

Per core c (h-slice c*512..(c+1)*512), per expert e (cnt_e tokens, exact):
  mm1: hT[hc*128:(hc+1)*128, tok] = relu(W1_slice^T @ x^T + b1)  (h on
       partitions, 4 h-chunks, contraction D=1024 via 8 chained matmuls)
  mm2: yT[dt*128:(dt+1)*128, tok] += W2_chunk^T @ hT_chunk       (d on
       partitions, 8 d-tiles, contraction 512 via 4 chained matmuls)
The host computes the gate/top-2 (replicated small gate), groups tokens
expert-major, sums the 8 partial yT outputs, applies the combine weight and
b2, and scatter-adds back to token order. Exactness: out = w*(y_dev) +
w*b2, so folding b2 on the host is exact.

Token chunks of <=512 (PSUM bank width) are software-pipelined: mm2 of
chunk i is emitted after mm1 of chunk i+1, so the PE never waits on the
scalar-engine relu. Two HWDGE queues (sync: x in + odd y out; gpsimd:
weights + even y out) so the startup fill is not serialized behind one
queue and the final two output chunks drain in parallel. bf16 keeps the
PE at full rate at any moving size and halves HBM traffic (~50 MB/core,
well under the compute shadow).
"""

import numpy as np
import ml_dtypes

P = 128
D = 1024
H = 4096
E = 8
TOPK = 2
DK = D // P        # 8 contraction chunks for mm1
HS = H // E        # 512 hidden units per core
HC = HS // P       # 4 h-chunks per core
DT = D // P        # 8 output d-tiles
CH = 512           # max token chunk (PSUM bank = 512 fp32)


def _chunk_items(cnts):
    """Split each expert's token count into chunks <=CH tokens.
    Returns [(e, global_off, tsz, first_of_e)], expert-major order.
    Expert 0 leads with a small chunk so the first matmul can start as soon
    as the first x block lands; the final two chunks are kept small so the
    last PSUM->SBUF->DRAM drains (on two queues) expose almost no tail."""
    items = []
    off = 0
    for e, cnt in enumerate(cnts):
        if cnt == 0:
            continue
        sizes = []
        rem = cnt
        if e == 0 and cnt > 360:
            sizes.append(224)
            rem -= 224
        n = -(-rem // CH)
        base, r = divmod(rem, n)
        sizes += [base + 1] * r + [base] * (n - r)
        for k, s in enumerate(sizes):
            items.append((e, off, s, k == 0))
            off += s
    e, o, s, f = items[-1]
    if s > 224 and not f:
        items[-1] = (e, o, s - 128, f)
        items.append((e, o + s - 128, 128, False))
    return items


def _build_program(cnts):
    import concourse.mybir as mybir
    import concourse.tile as tile
    from concourse import bacc

    f32 = mybir.dt.float32
    bf16 = mybir.dt.bfloat16
    Relu = mybir.ActivationFunctionType.Relu
    TOT = sum(cnts)
    items = _chunk_items(cnts)

    nc = bacc.Bacc(
        "TRN2",
        target_bir_lowering=False,
        debug=False,
        enable_asserts=True,
        num_devices=E,
    )
    xg_d = nc.dram_tensor("xg", [P, DK, TOT], bf16, kind="ExternalInput").ap()
    w1_d = nc.dram_tensor("w1", [P, DK, E * HS], bf16, kind="ExternalInput").ap()
    w2_d = nc.dram_tensor("w2", [P, E * HC, D], bf16, kind="ExternalInput").ap()
    b1_d = nc.dram_tensor("b1", [P, E * HC], f32, kind="ExternalInput").ap()
    y_d = nc.dram_tensor("y", [P, DT, TOT], bf16, kind="ExternalOutput").ap()

    with tile.TileContext(nc) as tc:
        with (
            tc.tile_pool(name="const", bufs=1) as const,
            tc.tile_pool(name="w1p", bufs=2) as w1p,
            tc.tile_pool(name="w2p", bufs=2) as w2p,
            tc.tile_pool(name="xgp", bufs=4) as xgp,
            tc.tile_pool(name="htp", bufs=3) as htp,
            tc.tile_pool(name="ysp", bufs=3) as ysp,
            tc.tile_pool(name="php", bufs=2, space="PSUM") as php,
            tc.tile_pool(name="pyp", bufs=6, space="PSUM") as pyp,
        ):
            def load_w1(e, quarters):
                # halves (quarters for the very first expert) so the first
                # h-chunk chains never wait on a 1MB descriptor
                t = w1p.tile([P, DK, HS], bf16, tag="w1")
                step = P if quarters else 2 * P
                for lo in range(0, HS, step):
                    nc.gpsimd.dma_start(
                        t[:, :, lo:lo + step],
                        w1_d[:, :, e * HS + lo:e * HS + lo + step],
                    )
                return t

            def load_w2(e):
                t = w2p.tile([P, HC, D], bf16, tag="w2")
                nc.gpsimd.dma_start(t[:, :, 0:D // 2], w2_d[:, e * HC:(e + 1) * HC, 0:D // 2])
                nc.gpsimd.dma_start(t[:, :, D // 2:D], w2_d[:, e * HC:(e + 1) * HC, D // 2:D])
                return t

            def load_xg(off, tsz):
                t = xgp.tile([P, DK, CH], bf16, tag="xg")
                nc.sync.dma_start(t[:, :, 0:tsz], xg_d[:, :, off:off + tsz])
                return t

            b1t = const.tile([P, E * HC], f32)
            w1_tiles = {}
            w2_tiles = {}
            ht_tiles = {}

            def mm1(i):
                e, off, tsz, first = items[i]
                xgt = load_xg(off, tsz)
                if first and e == items[0][0]:
                    nc.gpsimd.dma_start(b1t[:], b1_d[:])
                if first:
                    w2_tiles[e] = load_w2(e)
                w1t = w1_tiles[e]
                htt = htp.tile([P, HC, CH], bf16, tag="ht")
                ht_tiles[i] = htt
                for hc in range(HC):
                    ph = php.tile([P, CH], f32, tag="ph")
                    for dk in range(DK):
                        nc.tensor.matmul(
                            ph[:, 0:tsz],
                            w1t[:, dk, hc * P:(hc + 1) * P],
                            xgt[:, dk, 0:tsz],
                            start=(dk == 0),
                            stop=(dk == DK - 1),
                        )
                    nc.scalar.activation(
                        htt[:, hc, 0:tsz], ph[:, 0:tsz], Relu,
                        bias=b1t[:, e * HC + hc:e * HC + hc + 1],
                    )

            def mm2(i):
                e, off, tsz, first = items[i]
                htt = ht_tiles.pop(i)
                w2t = w2_tiles[e]
                yst = ysp.tile([P, DT, CH], bf16, tag="ys")
                for dt in range(DT):
                    py = pyp.tile([P, CH], f32, tag="py")
                    for hc in range(HC):
                        nc.tensor.matmul(
                            py[:, 0:tsz],
                            w2t[:, hc, dt * P:(dt + 1) * P],
                            htt[:, hc, 0:tsz],
                            start=(hc == 0),
                            stop=(hc == HC - 1),
                        )
                    if dt < 5:
                        nc.vector.tensor_copy(yst[:, dt, 0:tsz], py[:, 0:tsz])
                    else:
                        nc.scalar.copy(yst[:, dt, 0:tsz], py[:, 0:tsz])
                eng = nc.sync if (i % 2) else nc.gpsimd
                eng.dma_start(y_d[:, :, off:off + tsz], yst[:, :, 0:tsz])

            w1_tiles[items[0][0]] = load_w1(items[0][0], quarters=True)
            for i in range(len(items)):
                e, off, tsz, first = items[i]
                if first and e != items[0][0]:
                    w1_tiles[e] = load_w1(e, quarters=False)
                mm1(i)
                if i > 0:
                    mm2(i - 1)
            mm2(len(items) - 1)
    nc.compile()
    return nc, items


def _route(x, Wg, bg):
    """Host gate: softmax over experts + stable top-2 (mirrors jax.lax.top_k
    tie-breaking: lowest index first)."""
    logits = x @ Wg + bg
    mx = logits.max(axis=1, keepdims=True)
    ex = np.exp(logits - mx)
    gate = ex / ex.sum(axis=1, keepdims=True)
    top2 = np.argsort(-gate, axis=1, kind="stable")[:, :TOPK]
    return gate, top2


def kernel(x, Wg, bg, W1, b1, W2, b2):
    from concourse.bass_utils import run_bass_kernel_spmd

    bf = ml_dtypes.bfloat16
    x = np.asarray(x, np.float32)
    Wg = np.asarray(Wg, np.float32)
    bg = np.asarray(bg, np.float32)
    W1 = np.asarray(W1, np.float32)
    b1 = np.asarray(b1, np.float32)
    W2 = np.asarray(W2, np.float32)
    b2 = np.asarray(b2, np.float32)
    Ttok = x.shape[0]

    gate, top2 = _route(x, Wg, bg)
    expert_idx = [np.nonzero((top2 == e).any(axis=1))[0] for e in range(E)]
    cnts = [len(s) for s in expert_idx]
    TOT = sum(cnts)
    order = np.concatenate([s for s in expert_idx if len(s)])
    offs = np.cumsum([0] + cnts)

    nc, _items = _build_program(cnts)

    # xg: x^T gathered expert-major, D-chunk tiled: xg[p, dk, j] =
    # x[order[j], dk*128 + p].  Identical for every core.
    xg = np.ascontiguousarray(
        x[order].astype(bf).T.reshape(DK, P, TOT).transpose(1, 0, 2)
    )
    W1b = W1.astype(bf)
    W2b = W2.astype(bf)
    in_maps = []
    for c in range(E):
        # w1[p, dk, e*512 + h] = W1[e, dk*128+p, c*512+h]
        w1c = np.ascontiguousarray(
            W1b[:, :, c * HS:(c + 1) * HS]
            .reshape(E, DK, P, HS).transpose(2, 1, 0, 3).reshape(P, DK, E * HS)
        )
        # w2[p, e*4+hc, d] = W2[e, c*512 + hc*128 + p, d]
        w2c = np.ascontiguousarray(
            W2b[:, c * HS:(c + 1) * HS, :]
            .reshape(E, HC, P, D).transpose(2, 0, 1, 3).reshape(P, E * HC, D)
        )
        # b1s[p, e*4+hc] = b1[e, c*512 + hc*128 + p]
        b1c = np.ascontiguousarray(
            b1[:, c * HS:(c + 1) * HS].reshape(E, HC, P).transpose(2, 0, 1)
            .reshape(P, E * HC).astype(np.float32)
        )
        in_maps.append({"xg": xg, "w1": w1c, "w2": w2c, "b1": b1c})

    results = run_bass_kernel_spmd(nc, in_maps, core_ids=list(range(E))).results

    # Sum the 8 partial yT, apply combine weights, scatter back to tokens.
    acc = np.zeros((P, DT, TOT), np.float32)
    for c in range(E):
        acc += results[c]["y"].astype(np.float32)
    yT = acc.transpose(1, 0, 2).reshape(D, TOT)   # yT[d, j]
    out = np.zeros((Ttok, D), np.float32)
    for e in range(E):
        idx = expert_idx[e]
        if len(idx) == 0:
            continue
        blk = yT[:, offs[e]:offs[e + 1]].T
        out[idx] += gate[idx, e:e + 1] * blk
    # b2 contribution, folded on the host (exact: w*y device + w*b2 here)
    mask = np.zeros((Ttok, E), np.float32)
    np.put_along_axis(mask, top2, 1.0, axis=1)
    out += (gate * mask) @ b2
    return out
